# revision 52
# baseline (speedup 1.0000x reference)
"""MoE pre-activation residual block on 8 trn2 NeuronCores (expert-parallel).

kernel(**inputs) takes the full unsharded inputs (numpy, keyed as in
setup_inputs) and returns the full [N, D] float32 output.

Host: LayerNorm+relu, router logits, top-2 gating, capacity-based dispatch
      (builds expert_inputs per expert), final gather/combine/residual.
Device (one expert per core, SPMD): the expert MLP
      y = LN_h(x @ W1 + b1) -> relu -> @ W2 + b2
  computed as two bf16 matmuls with fp32 PSUM accumulation:
    - h^T[H, c] = sum_d W1'[d, h] x^T[d, c]  (lhsT = W1 as stored); the LN
      mean over H is folded into the weights on the host
      (W1' = W1 - rowmean_H(W1)), so PSUM holds h - mu directly
    - var = sum_H (h-mu)^2: ACT squares each PSUM tile (bf16), DVE folds the
      16 tiles with a pairwise add tree, PE does ONE ones-matmul reduction
    - hn = relu(h - mu) in bf16 straight from PSUM (rstd > 0 commutes w/
      relu, so it can be applied after mm2)
    - y^T[D, c] = sum_h W2[h, d] hn[h, c]; rstd (broadcast across partitions
      by GpSimd) applied at PSUM eviction on DVE
"""

import sys

try:
    import concourse.bacc  # noqa: F401
except ImportError:  # pragma: no cover
    for _p in ("/opt/trn_rl_repo", "/root/.axon_site/_ro/trn_rl_repo"):
        if _p not in sys.path:
            sys.path.append(_p)

import numpy as np
import ml_dtypes

import concourse.bacc as bacc
import concourse.mybir as mybir
import concourse.tile as tile
from concourse.bass_utils import run_bass_kernel_spmd

# ---------------------------------------------------------------- shim -----
# Under axon, run_bass_kernel_spmd(trace=True) needs antenv.axon_hooks for
# NTFF profiling. Some images lack it; register an equivalent hook so a
# BASS_TRACE=1 run still produces timing instead of silently skipping.
def _install_axon_hooks_shim():
    try:
        import antenv.axon_hooks  # noqa: F401
        return
    except ImportError:
        pass
    import contextlib, ctypes, types, os

    so = "/opt/axon/libaxon_pjrt.so"
    hook = None
    if os.path.exists(so):
        try:
            lib = ctypes.CDLL(so)
            if hasattr(lib, "axon_start_nrt_profile"):
                lib.axon_start_nrt_profile.argtypes = [
                    ctypes.POINTER(ctypes.c_int64),
                    ctypes.c_size_t,
                ]
                lib.axon_start_nrt_profile.restype = ctypes.c_int64
                lib.axon_stop_nrt_profile.argtypes = [ctypes.c_char_p]
                lib.axon_stop_nrt_profile.restype = ctypes.c_int64

                @contextlib.contextmanager
                def _hook(output_dir, device_ids):
                    import jax

                    jax.devices()
                    if device_ids:
                        ids = (ctypes.c_int64 * len(device_ids))(*device_ids)
                        rc = lib.axon_start_nrt_profile(ids, len(device_ids))
                    else:
                        rc = lib.axon_start_nrt_profile(None, 0)
                    if rc != 0:
                        raise RuntimeError(f"axon_start_nrt_profile rc={rc}")
                    try:
                        yield
                    finally:
                        n = lib.axon_stop_nrt_profile(str(output_dir).encode())
                        print(f"ntff profile: {n} file(s) -> {output_dir}",
                              file=sys.stderr)

                hook = _hook
        except OSError:
            hook = None
    mod = types.ModuleType("antenv.axon_hooks")
    mod.get_axon_ntff_profile_hook = lambda: hook
    mod.set_axon_ntff_profile_hook = lambda h: None
    sys.modules["antenv.axon_hooks"] = mod


_install_axon_hooks_shim()

# ------------------------------------------------------------- constants ---
N, D, H, E, TOPK = 16384, 1024, 2048, 8, 2
CAP = 4096
EPS = 1e-6
P = 128
C = 512                      # CAP-chunk (columns per pipeline step)
KD, KH = D // P, H // P      # 8 k-subtiles for mm1, 16 for mm2
MT = H // P                  # 16 output row-tiles of mm1 (H rows)
DT = D // P                  # 8 output row-tiles of mm2 (D rows)
NCH = CAP // C               # chunks

BF16 = mybir.dt.bfloat16
F32 = mybir.dt.float32
F8 = mybir.dt.float8e4
npbf16 = ml_dtypes.bfloat16
npf8 = ml_dtypes.float8_e4m3fn

# fp8 scale factors: keep W1/W2 out of the e4m3 subnormal range while
# keeping |h'| well below TRN's e4m3 Inf threshold (240). Both cancel
# automatically through the self-computed rstd except S2, which is folded
# into the Sqrt activation's scale (see _build).
S1 = 16.0
S2 = 32.0
F8CLIP = 240.0

import os
VARIANT = os.environ.get("BASS_VARIANT", "full8")  # full8 | mix8 | bf16

_nc_cache = {}


def _build(flags):
    """Build the per-core SPMD bass program.
    flags = (b1_nz, ns_nb_nz, b2_nz, mm1_fp8, mm2_fp8).

    The LayerNorm mean over H is folded into the weights on the host
    (W1' = W1 - rowmean_H(W1), b1' = b1 - mean(b1)), so PSUM holds h - mu
    directly after the W1' matmul.

    fp8 mode (fast path only): weights/activations quantized to e4m3 and
    matmuls run in DoubleRow perf mode (2 k-subtiles per instruction, 2x PE
    throughput). W1 is pre-scaled by S1 and W2 by S2 on the host; S1 (and
    the x-side scale) cancels through the self-computed rstd, S2 is folded
    into the Sqrt activation scale so rstd_used = rstd/S2."""
    b1_nz, ns_nb_nz, b2_nz, mm1_fp8, mm2_fp8 = flags
    fast = not (b1_nz or ns_nb_nz or b2_nz)
    nc = bacc.Bacc("TRN2", target_bir_lowering=False)

    XDT = F8 if mm1_fp8 else BF16
    W2DT = F8 if mm2_fp8 else BF16
    XP = KD // 2   # x k-pair tiles per chunk
    HP = KH // 2   # hn k-pair tiles per chunk
    # x and w1 come in pre-tiled host layouts so every DMA line is a
    # contiguous 1KB run per partition (vs 128B strided slices of the
    # canonical [D, *] layouts, which made kernel-startup DMA descriptor-
    # bound)
    xT_d = nc.dram_tensor("xT", [NCH, XP, P, 2, C], XDT, kind="ExternalInput")
    w1_d = nc.dram_tensor("w1", [MT, P, KD, P], XDT, kind="ExternalInput")
    w2_d = nc.dram_tensor("w2", [H, D], W2DT, kind="ExternalInput")
    if fast:
        # fast path emits token-major bf16 output (mm2 runs "transposed"
        # with hn as the stationary operand so rstd becomes a per-partition
        # scalar)
        y_d = nc.dram_tensor("y", [CAP, D], BF16, kind="ExternalOutput")
        y_r = y_d.rearrange("(ct p) d -> p ct d", p=P)
    else:
        yT_d = nc.dram_tensor("yT", [D, CAP], F32, kind="ExternalOutput")
        yT_r = yT_d.rearrange("(dt p) c -> p dt c", p=P)
    if b1_nz:
        b1_d = nc.dram_tensor("b1", [H, 1], BF16, kind="ExternalInput")
    if ns_nb_nz:
        nsc_d = nc.dram_tensor("nsc", [H, 1], F32, kind="ExternalInput")
        nbs_d = nc.dram_tensor("nbs", [H, 1], F32, kind="ExternalInput")
    if b2_nz:
        b2_d = nc.dram_tensor("b2", [D, 1], F32, kind="ExternalInput")

    xT_r = xT_d.rearrange("c j p v f -> p c j v f")
    w1_r = w1_d.rearrange("mt p k i -> p mt k i")
    w2_r = w2_d.rearrange("(ko p) d -> p ko d", p=P)

    with tile.TileContext(nc) as tc:
        with (
            tc.tile_pool(name="const", bufs=1) as cpool,
            tc.tile_pool(name="xp", bufs=3) as xpool,
            tc.tile_pool(name="hnp", bufs=2) as hnpool,
            tc.tile_pool(name="sqp", bufs=4) as sqpool,
            tc.tile_pool(name="rows", bufs=3) as rowpool,
            tc.tile_pool(name="rbp", bufs=2) as rbpool,
            tc.tile_pool(name="yp", bufs=3) as ypool,
            tc.tile_pool(name="hgen", bufs=1) as hgenpool,
            tc.tile_pool(name="ps_h", bufs=3, space="PSUM") as ps_h,
            tc.tile_pool(name="ps_y", bufs=4, space="PSUM") as ps_y,
            tc.tile_pool(name="ps_s", bufs=1, space="PSUM") as ps_s,
        ):
            # ---- resident constants. W1/x/hn are split into per-consumer
            # tiles (per mm1 row-tile / per k-pair) so Tile's dependency
            # tracking stays fine-grained: the first mm2 matmul of a chunk
            # only waits for its own two hn row-tiles, not all 16, which
            # keeps the PE from idling (and from dropping out of max
            # p-state) at the mm1->mm2 boundary. ---------------------------
            # Each dma_start costs ~600ns of serial Sync-queue issue time, so
            # inputs are loaded with FEW, large transfers: one per x chunk,
            # and w1 in 4 groups of 4 row-tiles (enough granularity that the
            # first mm1 group starts after ~0.5MB instead of the full 2MB).
            WG = 4
            x_tiles = [None] * NCH

            def emit_x_load(c):
                x_tiles[c] = xpool.tile([P, XP, 2, C], XDT, tag="x", name="x")
                nc.sync.dma_start(x_tiles[c][:], xT_r[:, c])

            # bulk DMA only starts flowing ~8us into the kernel and drains
            # roughly in issue order at ~200GB/s, so: x0 first, then w1 with
            # fine granularity up front (separate single row-tile tiles, so
            # whole-tile dependency tracking lets mm1 start as soon as
            # ~0.64MB has landed) and 2-row-tile groups after (each lands in
            # ~1.3us, ahead of the PE's 1.7us consumption pace).
            w1_first = [
                cpool.tile([P, KD, P], XDT, tag=f"w1f_{mt}", name="w1")
                for mt in range(WG)
            ]
            PG = 2
            w1_tiles = [None, None] + [
                cpool.tile([P, PG, KD, P], XDT, tag=f"w1_{g}", name="w1")
                for g in range(2, MT // PG)
            ]
            emit_x_load(0)
            for mt in range(WG):
                nc.sync.dma_start(w1_first[mt][:], w1_r[:, mt])
            for g in range(2, MT // PG):
                nc.sync.dma_start(w1_tiles[g][:], w1_r[:, g * PG:(g + 1) * PG])

            def w1_slice(mt, ksl):
                if mt < WG:
                    return w1_first[mt][:, ksl, :]
                return w1_tiles[mt // PG][:, mt % PG, ksl, :]
            ones_kcol = cpool.tile([P, 1], BF16, tag="ones_kcol", name="ones_kcol")
            nc.vector.memset(ones_kcol[:], 1.0)
            # PE p-state warm-up: ~24 trivial matmuls that only depend on the
            # ones_kcol memset run during the initial DMA wait, so the PE is
            # already at max clock when the first real mm1 group issues.
            if fast:
                warm_ps = ps_s.tile([P, C // P], F32, tag="sst", name="warm")
                for _ in range(120):
                    nc.tensor.matmul(warm_ps[:1, :1], lhsT=ones_kcol[:],
                                     rhs=ones_kcol[:], start=True, stop=True,
                                     skip_group_check=True)
            ones_krow_f = cpool.tile([1, P], F32, tag="ones_krow_f", name="ones_krow_f")
            nc.vector.memset(ones_krow_f[:], 1.0)
            eps_sb = cpool.tile([P, 1], F32, tag="eps", name="eps")
            nc.vector.memset(eps_sb[:], EPS)
            if b1_nz:
                b1_sb = cpool.tile([1, H], BF16, tag="b1", name="b1")
                nc.sync.dma_start(b1_sb[:], b1_d.rearrange("h x -> x h"))
                ones_row = cpool.tile([1, C], BF16, tag="ones_row", name="ones_row")
                nc.vector.memset(ones_row[:], 1.0)
            if ns_nb_nz:
                nsc_sb = cpool.tile([P, MT], F32, tag="nsc", name="nsc")
                nc.sync.dma_start(nsc_sb[:], nsc_d.rearrange("(mt p) x -> p mt x", p=P)[:, :, 0])
                nbs_sb = cpool.tile([P, MT], F32, tag="nbs", name="nbs")
                nc.sync.dma_start(nbs_sb[:], nbs_d.rearrange("(mt p) x -> p mt x", p=P)[:, :, 0])
            if b2_nz:
                b2_sb = cpool.tile([P, DT], F32, tag="b2", name="b2")
                nc.sync.dma_start(b2_sb[:], b2_d.rearrange("(dt p) x -> p dt x", p=P)[:, :, 0])
            # w2 loads are deferred into chunk 0's mm1 phase: issuing them at
            # t=0 steals HBM bandwidth from the w1/x transfers that gate mm1
            # startup, while w2 isn't needed until the first mm2 (~27us in)
            w2_sb = cpool.tile([P, KH, D], W2DT, tag="w2", name="w2")

            for c in range(NCH):
                xt = x_tiles[c]
                hn = [
                    hnpool.tile([P, 2, C], F8 if mm2_fp8 else BF16,
                                tag=f"hn{j}", name="hn")
                    for j in range(HP)
                ]
                hflat = hgenpool.tile([P, KH, C], F32, tag="hflat", name="hflat") if ns_nb_nz else None
                # mm1: 16 row-tile groups. ACT squares each PSUM tile (bf16);
                # the 16 squared tiles are folded with two sequential
                # accumulators — even leaves on DVE, odd leaves on the
                # otherwise-idle GpSimd — plus one final merge add, so the
                # post-mm1 tail is a single square + one add. Relus (the hn
                # eviction mm2 consumes) go 12 on DVE / 4 on ACT to keep
                # both engines under the PE's per-chunk budget.
                sq_tiles = [None] * MT
                accA = accB = None
                for mt in range(MT):
                    ph = ps_h.tile([P, C], F32, tag="ph", name="ph")
                    if mm1_fp8:
                        for kt in range(0, KD, 2):
                            nc.tensor.matmul(
                                ph[:], lhsT=w1_slice(mt, slice(kt, kt + 2)),
                                rhs=xt[:, kt // 2], start=(kt == 0),
                                stop=(kt == KD - 2 and not b1_nz),
                                perf_mode=mybir.MatmulPerfMode.DoubleRow,
                            )
                    else:
                        for kt in range(KD):
                            nc.tensor.matmul(
                                ph[:], lhsT=w1_slice(mt, kt),
                                rhs=xt[:, kt // 2, kt % 2, :], start=(kt == 0),
                                stop=(kt == KD - 1 and not b1_nz),
                            )
                    if b1_nz:
                        nc.tensor.matmul(
                            ph[:], lhsT=b1_sb[:, mt * P:(mt + 1) * P], rhs=ones_row[:],
                            start=False, stop=True, skip_group_check=True,
                        )
                    if ns_nb_nz:
                        nc.vector.tensor_copy(hflat[:, mt, :], ph[:])
                    elif mt % 8 == 7:
                        nc.scalar.activation(
                            hn[mt // 2][:, mt % 2, :], ph[:],
                            mybir.ActivationFunctionType.Relu,
                        )
                    else:
                        nc.vector.tensor_scalar_max(hn[mt // 2][:, mt % 2, :], ph[:], 0.0)
                    sq = sqpool.tile([P, C], BF16, tag="sq4", name="sq4")
                    sq_tiles[mt] = sq
                    nc.scalar.square(sq[:], ph[:])
                    if c == 0:
                        # deferred w2 loads, one k-tile per mm1 row-tile
                        nc.sync.dma_start(w2_sb[:, mt, :], w2_r[:, mt, :])
                    # even leaves 0-14 fold on the (otherwise idle, slower)
                    # GpSimd which goes quiet by mm1's end; odd leaves 1-13 on
                    # DVE; the two accumulators merge at ~mm1 end and sq15
                    # folds LAST, so only one DVE add trails the final square
                    # and the stats matmuls never stall the PE.
                    if mt == 2:
                        accB = sqpool.tile([P, C], BF16, tag="accB", name="accB")
                        nc.gpsimd.tensor_add(accB[:], sq_tiles[0][:], sq_tiles[2][:])
                    elif mt == 3:
                        accA = sqpool.tile([P, C], BF16, tag="accA", name="accA")
                        nc.vector.tensor_add(accA[:], sq_tiles[1][:], sq_tiles[3][:])
                    elif mt > 3 and mt < 15 and mt % 2 == 0:
                        nc.gpsimd.tensor_add(accB[:], accB[:], sq[:])
                    elif mt > 3 and mt < 15:
                        nc.vector.tensor_add(accA[:], accA[:], sq[:])
                merge0 = sqpool.tile([P, C], BF16, tag="mrg", name="mrg")
                nc.vector.tensor_add(merge0[:], accA[:], accB[:])
                hacc_bf = sqpool.tile([P, C], BF16, tag="hacc", name="hacc")
                nc.vector.tensor_add(hacc_bf[:], merge0[:], sq_tiles[15][:])

                if c + 1 < NCH:
                    emit_x_load(c + 1)

                def emit_stats_head(ss):
                    # (general path only) ss[1, C] = sum_p hacc_bf -> std ->
                    # rstd (row ops; the [1, C] reciprocal is a slow
                    # single-lane DVE op ~3.3us).
                    nc.tensor.matmul(ss[:1, :], lhsT=ones_kcol[:], rhs=hacc_bf[:],
                                     start=True, stop=True, skip_group_check=True)
                    std = rowpool.tile([1, C], F32, tag="std", name="std")
                    nc.scalar.activation(
                        std[:], ss[:1, :], mybir.ActivationFunctionType.Sqrt,
                        bias=eps_sb[:1, :], scale=1.0 / H,
                    )
                    rstd = rowpool.tile([1, C], F32, tag="rstd", name="rstd")
                    nc.vector.reciprocal(rstd[:], std[:])
                    return rstd

                def emit_rb(rstd):
                    # broadcast rstd across partitions on the (idle) GpSimd
                    rb = rbpool.tile([P, C], F32, tag="rb", name="rb")
                    nc.gpsimd.partition_broadcast(rb[:], rstd[:], channels=P)
                    return rb

                if ns_nb_nz:
                    # general path: hn = relu(((h-mu)*rstd)*nsc + nbs)
                    ss = ps_s.tile([P, C], F32, tag="small", name="small")
                    rstd = emit_stats_head(ss)
                    rb = emit_rb(rstd)
                    for mt in range(MT):
                        tmp = rbpool.tile([P, C], F32, tag="tmpn", name="tmpn")
                        nc.vector.tensor_mul(tmp[:], hflat[:, mt, :], rb[:])
                        nc.scalar.activation(
                            hn[mt // 2][:, mt % 2, :], tmp[:],
                            mybir.ActivationFunctionType.Relu,
                            bias=nbs_sb[:, mt, None], scale=nsc_sb[:, mt, None],
                        )

                    for dt in range(DT):
                        py = ps_y.tile([P, C], F32, tag="py", name="py")
                        for kt in range(KH):
                            nc.tensor.matmul(
                                py[:], lhsT=w2_sb[:, kt, dt * P:(dt + 1) * P],
                                rhs=hn[kt // 2][:, kt % 2, :], start=(kt == 0),
                                stop=(kt == KH - 1),
                            )
                        ysb = ypool.tile([P, C], F32, tag="y", name="y")
                        nc.vector.tensor_copy(ysb[:], py[:])
                        if b2_nz:
                            nc.vector.tensor_scalar_add(ysb[:], ysb[:], b2_sb[:, dt, None])
                        nc.sync.dma_start(yT_r[:, dt, c * C:(c + 1) * C], ysb[:])
                else:
                    # fast path: mm2 runs "transposed" — hn column-slices are
                    # the stationary operand, W2 the moving one, so the PSUM
                    # output is [c-subtile(128), d] and rstd is a per-PARTITION
                    # scalar. The stats reduce to 4 tiny [128,1] ones-matmuls
                    # + [128,4] sqrt/reciprocal (sub-us, vs ~4us for the
                    # single-lane [1,C] row chain), evictions are
                    # tensor_scalar_muls, and no partition-broadcast is
                    # needed. rstd > 0 commutes with relu so it can be
                    # applied after mm2; W2's host-side fp8 scale S2 is
                    # folded into the Sqrt scale: std' = S2*std, so the
                    # eviction multiply by 1/std' also divides out S2. The
                    # x/W1 scales cancel automatically (rstd is computed
                    # from h' itself).
                    CS = C // P   # 4 column-subtiles per chunk
                    DH = D // C   # 2 moving halves of D per (cs)
                    sqrt_scale = (S2 * S2 / H) if mm2_fp8 else (1.0 / H)
                    pys = [[None] * DH for _ in range(CS)]

                    def y_mms(cs, dh):
                        pys[cs][dh] = ps_y.tile([P, C], F32, tag="py", name="py")
                        if mm2_fp8:
                            for kt in range(0, KH, 2):
                                nc.tensor.matmul(
                                    pys[cs][dh][:],
                                    lhsT=hn[kt // 2][:, :, cs * P:(cs + 1) * P],
                                    rhs=w2_sb[:, kt:kt + 2, dh * C:(dh + 1) * C],
                                    start=(kt == 0), stop=(kt == KH - 2),
                                    perf_mode=mybir.MatmulPerfMode.DoubleRow,
                                )
                        else:
                            for kt in range(KH):
                                nc.tensor.matmul(
                                    pys[cs][dh][:],
                                    lhsT=hn[kt // 2][:, kt % 2, cs * P:(cs + 1) * P],
                                    rhs=w2_sb[:, kt, dh * C:(dh + 1) * C],
                                    start=(kt == 0), stop=(kt == KH - 1),
                                )

                    def emit_stats_t():
                        # ss_t[:, cs] = sum_h h'^2 for column-subtile cs
                        ss_t = ps_s.tile([P, CS], F32, tag="sst", name="sst")
                        for cs in range(CS):
                            nc.tensor.matmul(
                                ss_t[:, cs:cs + 1],
                                lhsT=hacc_bf[:, cs * P:(cs + 1) * P],
                                rhs=ones_kcol[:], start=True, stop=True,
                                skip_group_check=True,
                            )
                        std_t = rowpool.tile([P, CS], F32, tag="stdt", name="stdt")
                        nc.scalar.activation(
                            std_t[:], ss_t[:],
                            mybir.ActivationFunctionType.Sqrt,
                            bias=eps_sb[:], scale=sqrt_scale,
                        )
                        rstd_t = rowpool.tile([P, CS], F32, tag="rstdt", name="rstdt")
                        nc.vector.reciprocal(rstd_t[:], std_t[:])
                        return rstd_t

                    def y_evict(cs, dh, rstd_t):
                        ysb = ypool.tile([P, C], BF16, tag="y", name="y")
                        nc.vector.tensor_scalar_mul(
                            ysb[:], pys[cs][dh][:], rstd_t[:, cs, None]
                        )
                        nc.sync.dma_start(
                            y_r[:, c * CS + cs, dh * C:(dh + 1) * C], ysb[:]
                        )

                    y_mms(0, 0)
                    emitted_stats = False
                    rstd_t = None
                    for i, (cs, dh) in enumerate(
                        [(cs, dh) for cs in range(CS) for dh in range(DH)][1:]
                    ):
                        y_mms(cs, dh)
                        if not emitted_stats:
                            rstd_t = emit_stats_t()
                            emitted_stats = True
                            y_evict(0, 0, rstd_t)
                        else:
                            prev = (cs * DH + dh) - 1
                            y_evict(prev // DH, prev % DH, rstd_t)
                    y_evict(CS - 1, DH - 1, rstd_t)

    nc.compile()
    return nc


# ------------------------------------------------------------ host logic ---
def _route(x0, ln_scale, ln_bias, Wr, br):
    """LayerNorm -> relu -> router logits -> top-2 -> gates (float64 math)."""
    x = x0.astype(np.float64)
    mu = x.mean(axis=-1, keepdims=True)
    var = np.square(x - mu).mean(axis=-1, keepdims=True)
    xn = (x - mu) / np.sqrt(var + EPS)
    xn = xn * ln_scale.astype(np.float64) + ln_bias.astype(np.float64)
    np.maximum(xn, 0.0, out=xn)
    logits = xn @ Wr.astype(np.float64) + br.astype(np.float64)

    n = logits.shape[0]
    rows = np.arange(n)
    i0 = np.argmax(logits, axis=1)
    l0 = logits[rows, i0]
    tmp = logits.copy()
    tmp[rows, i0] = -np.inf
    i1 = np.argmax(tmp, axis=1)
    l1 = tmp[rows, i1]
    # softmax over (l0, l1); l0 >= l1
    e1 = np.exp(l1 - l0)
    g0 = 1.0 / (1.0 + e1)
    g1 = e1 / (1.0 + e1)
    top_idx = np.stack([i0, i1], axis=1).astype(np.int64)
    gates = np.stack([g0, g1], axis=1)
    return xn.astype(np.float32), top_idx, gates


def _positions(top_idx):
    """Capacity positions: running per-expert count in token-major slot order."""
    eidx = top_idx.reshape(-1)
    nk = eidx.shape[0]
    oh = (eidx[:, None] == np.arange(E)[None, :]).astype(np.int64)
    pos = np.cumsum(oh, axis=0)[np.arange(nk), eidx] - 1
    mask = pos < CAP
    pos_c = np.minimum(pos, CAP - 1)
    return eidx, pos, pos_c, mask


def kernel(**inputs):
    x0 = np.asarray(inputs["x0"], np.float32)
    ln_scale = np.asarray(inputs["ln_scale"], np.float32)
    ln_bias = np.asarray(inputs["ln_bias"], np.float32)
    Wr = np.asarray(inputs["Wr"], np.float32)
    br = np.asarray(inputs["br"], np.float32)
    W1 = np.asarray(inputs["W1"], np.float32)
    b1 = np.asarray(inputs["b1"], np.float32)
    n_scale = np.asarray(inputs["n_scale"], np.float32)
    n_bias = np.asarray(inputs["n_bias"], np.float32)
    W2 = np.asarray(inputs["W2"], np.float32)
    b2 = np.asarray(inputs["b2"], np.float32)

    # ---- host routing + dispatch ---------------------------------------
    xn, top_idx, gates = _route(x0, ln_scale, ln_bias, Wr, br)
    eidx, pos, pos_c, mask = _positions(top_idx)

    tok_of_slot = np.repeat(np.arange(N), TOPK)
    keep = mask
    expert_inputs = np.zeros((E, CAP, D), np.float32)
    expert_inputs[eidx[keep], pos[keep]] = xn[tok_of_slot[keep]]

    # ---- build / fetch compiled program --------------------------------
    b1_nz = bool(np.any(b1))
    ns_nb_nz = bool(np.any(n_scale != 1.0) or np.any(n_bias))
    b2_nz = bool(np.any(b2))
    # fp8 DoubleRow only wired up for the fast path (all-zero biases /
    # identity norm, which is what this problem's inputs always are)
    fast = not (b1_nz or ns_nb_nz or b2_nz)
    mm1_fp8 = fast and VARIANT in ("full8", "mix8")
    mm2_fp8 = fast and VARIANT == "full8"
    flags = (b1_nz, ns_nb_nz, b2_nz, mm1_fp8, mm2_fp8)
    if flags not in _nc_cache:
        _nc_cache[flags] = _build(flags)
    nc = _nc_cache[flags]

    def to_f8(a):
        return np.clip(a, -F8CLIP, F8CLIP).astype(npf8)

    XP = KD // 2

    def tile_x(xT):
        # [D, CAP] -> [NCH, XP, P, 2, C]: per (chunk, k-pair) tiles whose
        # per-partition rows are contiguous 2*C runs (efficient DMA lines)
        a = xT.reshape(XP, 2, P, NCH, C)          # [j, v, p, c, cc]
        return np.ascontiguousarray(a.transpose(3, 0, 2, 1, 4))

    def tile_w1(w1p):
        # [D, H] -> [MT, P, KD, P]: per mm1-row-tile slices, contiguous rows
        a = w1p.reshape(KD, P, MT, P)             # [ko, p, mt, i]
        return np.ascontiguousarray(a.transpose(2, 1, 0, 3))

    # ---- per-core inputs ----------------------------------------------
    in_maps = []
    for e in range(E):
        # Fold the LayerNorm mean over H into the weights: x @ W1' = h - mu.
        w1p = W1[e].astype(np.float64)
        w1p = w1p - w1p.mean(axis=1, keepdims=True)
        xTe = np.ascontiguousarray(expert_inputs[e].T)
        m = {
            "xT": tile_x(to_f8(xTe) if mm1_fp8 else xTe.astype(npbf16)),
            "w1": tile_w1(to_f8(w1p * S1) if mm1_fp8 else w1p.astype(npbf16)),
            "w2": to_f8(W2[e] * S2) if mm2_fp8 else W2[e].astype(npbf16),
        }
        if b1_nz:
            b1p = b1[e].astype(np.float64)
            b1p = b1p - b1p.mean()
            m["b1"] = b1p.astype(npbf16)[:, None]
        if ns_nb_nz:
            m["nsc"] = n_scale[e].astype(np.float32)[:, None]
            m["nbs"] = n_bias[e].astype(np.float32)[:, None]
        if b2_nz:
            m["b2"] = b2[e].astype(np.float32)[:, None]
        in_maps.append(m)

    res = run_bass_kernel_spmd(nc, in_maps, core_ids=list(range(E)))

    # ---- combine -------------------------------------------------------
    w = (gates.astype(np.float32) * mask.reshape(N, TOPK))
    pos2 = pos_c.reshape(N, TOPK)
    mix = np.zeros((N, D), np.float32)
    if fast:
        y_all = np.stack([res.results[e]["y"] for e in range(E)])  # [E, CAP, D] bf16
        for k in range(TOPK):
            mix += y_all[top_idx[:, k], pos2[:, k], :].astype(np.float32) * w[:, k:k + 1]
    else:
        yT_all = np.stack([res.results[e]["yT"] for e in range(E)])  # [E, D, CAP]
        for k in range(TOPK):
            mix += yT_all[top_idx[:, k], :, pos2[:, k]] * w[:, k:k + 1]
    return x0 + mix



# revision 53
# speedup vs baseline: 1.0162x; 1.0162x over previous
"""MoE pre-activation residual block on 8 trn2 NeuronCores (expert-parallel).

kernel(**inputs) takes the full unsharded inputs (numpy, keyed as in
setup_inputs) and returns the full [N, D] float32 output.

Host: LayerNorm+relu, router logits, top-2 gating, capacity-based dispatch
      (builds expert_inputs per expert), final gather/combine/residual.
Device (one expert per core, SPMD): the expert MLP
      y = LN_h(x @ W1 + b1) -> relu -> @ W2 + b2
  computed as two bf16 matmuls with fp32 PSUM accumulation:
    - h^T[H, c] = sum_d W1'[d, h] x^T[d, c]  (lhsT = W1 as stored); the LN
      mean over H is folded into the weights on the host
      (W1' = W1 - rowmean_H(W1)), so PSUM holds h - mu directly
    - var = sum_H (h-mu)^2: ACT squares each PSUM tile (bf16), DVE folds the
      16 tiles with a pairwise add tree, PE does ONE ones-matmul reduction
    - hn = relu(h - mu) in bf16 straight from PSUM (rstd > 0 commutes w/
      relu, so it can be applied after mm2)
    - y^T[D, c] = sum_h W2[h, d] hn[h, c]; rstd (broadcast across partitions
      by GpSimd) applied at PSUM eviction on DVE
"""

import sys

try:
    import concourse.bacc  # noqa: F401
except ImportError:  # pragma: no cover
    for _p in ("/opt/trn_rl_repo", "/root/.axon_site/_ro/trn_rl_repo"):
        if _p not in sys.path:
            sys.path.append(_p)

import numpy as np
import ml_dtypes

import concourse.bacc as bacc
import concourse.mybir as mybir
import concourse.tile as tile
from concourse.bass_utils import run_bass_kernel_spmd

# ---------------------------------------------------------------- shim -----
# Under axon, run_bass_kernel_spmd(trace=True) needs antenv.axon_hooks for
# NTFF profiling. Some images lack it; register an equivalent hook so a
# BASS_TRACE=1 run still produces timing instead of silently skipping.
def _install_axon_hooks_shim():
    try:
        import antenv.axon_hooks  # noqa: F401
        return
    except ImportError:
        pass
    import contextlib, ctypes, types, os

    so = "/opt/axon/libaxon_pjrt.so"
    hook = None
    if os.path.exists(so):
        try:
            lib = ctypes.CDLL(so)
            if hasattr(lib, "axon_start_nrt_profile"):
                lib.axon_start_nrt_profile.argtypes = [
                    ctypes.POINTER(ctypes.c_int64),
                    ctypes.c_size_t,
                ]
                lib.axon_start_nrt_profile.restype = ctypes.c_int64
                lib.axon_stop_nrt_profile.argtypes = [ctypes.c_char_p]
                lib.axon_stop_nrt_profile.restype = ctypes.c_int64

                @contextlib.contextmanager
                def _hook(output_dir, device_ids):
                    import jax

                    jax.devices()
                    if device_ids:
                        ids = (ctypes.c_int64 * len(device_ids))(*device_ids)
                        rc = lib.axon_start_nrt_profile(ids, len(device_ids))
                    else:
                        rc = lib.axon_start_nrt_profile(None, 0)
                    if rc != 0:
                        raise RuntimeError(f"axon_start_nrt_profile rc={rc}")
                    try:
                        yield
                    finally:
                        n = lib.axon_stop_nrt_profile(str(output_dir).encode())
                        print(f"ntff profile: {n} file(s) -> {output_dir}",
                              file=sys.stderr)

                hook = _hook
        except OSError:
            hook = None
    mod = types.ModuleType("antenv.axon_hooks")
    mod.get_axon_ntff_profile_hook = lambda: hook
    mod.set_axon_ntff_profile_hook = lambda h: None
    sys.modules["antenv.axon_hooks"] = mod


_install_axon_hooks_shim()

# ------------------------------------------------------------- constants ---
N, D, H, E, TOPK = 16384, 1024, 2048, 8, 2
CAP = 4096
EPS = 1e-6
P = 128
C = 512                      # CAP-chunk (columns per pipeline step)
KD, KH = D // P, H // P      # 8 k-subtiles for mm1, 16 for mm2
MT = H // P                  # 16 output row-tiles of mm1 (H rows)
DT = D // P                  # 8 output row-tiles of mm2 (D rows)
NCH = CAP // C               # chunks

BF16 = mybir.dt.bfloat16
F32 = mybir.dt.float32
F8 = mybir.dt.float8e4
npbf16 = ml_dtypes.bfloat16
npf8 = ml_dtypes.float8_e4m3fn

# fp8 scale factors: keep W1/W2 out of the e4m3 subnormal range while
# keeping |h'| well below TRN's e4m3 Inf threshold (240). Both cancel
# automatically through the self-computed rstd except S2, which is folded
# into the Sqrt activation's scale (see _build).
S1 = 16.0
S2 = 32.0
F8CLIP = 240.0

import os
VARIANT = os.environ.get("BASS_VARIANT", "full8")  # full8 | mix8 | bf16

_nc_cache = {}


def _build(flags):
    """Build the per-core SPMD bass program.
    flags = (b1_nz, ns_nb_nz, b2_nz, mm1_fp8, mm2_fp8).

    The LayerNorm mean over H is folded into the weights on the host
    (W1' = W1 - rowmean_H(W1), b1' = b1 - mean(b1)), so PSUM holds h - mu
    directly after the W1' matmul.

    fp8 mode (fast path only): weights/activations quantized to e4m3 and
    matmuls run in DoubleRow perf mode (2 k-subtiles per instruction, 2x PE
    throughput). W1 is pre-scaled by S1 and W2 by S2 on the host; S1 (and
    the x-side scale) cancels through the self-computed rstd, S2 is folded
    into the Sqrt activation scale so rstd_used = rstd/S2."""
    b1_nz, ns_nb_nz, b2_nz, mm1_fp8, mm2_fp8 = flags
    fast = not (b1_nz or ns_nb_nz or b2_nz)
    nc = bacc.Bacc("TRN2", target_bir_lowering=False)

    XDT = F8 if mm1_fp8 else BF16
    W2DT = F8 if mm2_fp8 else BF16
    XP = KD // 2   # x k-pair tiles per chunk
    HP = KH // 2   # hn k-pair tiles per chunk
    # x and w1 come in pre-tiled host layouts so every DMA line is a
    # contiguous 1KB run per partition (vs 128B strided slices of the
    # canonical [D, *] layouts, which made kernel-startup DMA descriptor-
    # bound)
    xT_d = nc.dram_tensor("xT", [NCH, XP, P, 2, C], XDT, kind="ExternalInput")
    w1_d = nc.dram_tensor("w1", [MT, P, KD, P], XDT, kind="ExternalInput")
    w2_d = nc.dram_tensor("w2", [H, D], W2DT, kind="ExternalInput")
    if fast:
        # fast path emits token-major bf16 output (mm2 runs "transposed"
        # with hn as the stationary operand so rstd becomes a per-partition
        # scalar)
        y_d = nc.dram_tensor("y", [CAP, D], BF16, kind="ExternalOutput")
        y_r = y_d.rearrange("(ct p) d -> p ct d", p=P)
    else:
        yT_d = nc.dram_tensor("yT", [D, CAP], F32, kind="ExternalOutput")
        yT_r = yT_d.rearrange("(dt p) c -> p dt c", p=P)
    if b1_nz:
        b1_d = nc.dram_tensor("b1", [H, 1], BF16, kind="ExternalInput")
    if ns_nb_nz:
        nsc_d = nc.dram_tensor("nsc", [H, 1], F32, kind="ExternalInput")
        nbs_d = nc.dram_tensor("nbs", [H, 1], F32, kind="ExternalInput")
    if b2_nz:
        b2_d = nc.dram_tensor("b2", [D, 1], F32, kind="ExternalInput")

    xT_r = xT_d.rearrange("c j p v f -> p c j v f")
    w1_r = w1_d.rearrange("mt p k i -> p mt k i")
    w2_r = w2_d.rearrange("(ko p) d -> p ko d", p=P)

    with tile.TileContext(nc) as tc:
        with (
            tc.tile_pool(name="const", bufs=1) as cpool,
            tc.tile_pool(name="xp", bufs=3) as xpool,
            tc.tile_pool(name="hnp", bufs=2) as hnpool,
            tc.tile_pool(name="sqp", bufs=4) as sqpool,
            tc.tile_pool(name="rows", bufs=3) as rowpool,
            tc.tile_pool(name="rbp", bufs=2) as rbpool,
            tc.tile_pool(name="yp", bufs=3) as ypool,
            tc.tile_pool(name="hgen", bufs=1) as hgenpool,
            tc.tile_pool(name="ps_h", bufs=3, space="PSUM") as ps_h,
            tc.tile_pool(name="ps_y", bufs=4, space="PSUM") as ps_y,
            tc.tile_pool(name="ps_s", bufs=1, space="PSUM") as ps_s,
        ):
            # ---- resident constants. W1/x/hn are split into per-consumer
            # tiles (per mm1 row-tile / per k-pair) so Tile's dependency
            # tracking stays fine-grained: the first mm2 matmul of a chunk
            # only waits for its own two hn row-tiles, not all 16, which
            # keeps the PE from idling (and from dropping out of max
            # p-state) at the mm1->mm2 boundary. ---------------------------
            # Each dma_start costs ~600ns of serial Sync-queue issue time, so
            # inputs are loaded with FEW, large transfers: one per x chunk,
            # and w1 in 4 groups of 4 row-tiles (enough granularity that the
            # first mm1 group starts after ~0.5MB instead of the full 2MB).
            WG = 4
            x_tiles = [None] * NCH

            def emit_x_load(c):
                x_tiles[c] = xpool.tile([P, XP, 2, C], XDT, tag="x", name="x")
                nc.sync.dma_start(x_tiles[c][:], xT_r[:, c])

            # bulk DMA only starts flowing ~8us into the kernel and drains
            # roughly in issue order at ~200GB/s, so: x0 first, then w1 with
            # fine granularity up front (separate single row-tile tiles, so
            # whole-tile dependency tracking lets mm1 start as soon as
            # ~0.64MB has landed) and 2-row-tile groups after (each lands in
            # ~1.3us, ahead of the PE's 1.7us consumption pace).
            w1_first = [
                cpool.tile([P, KD, P], XDT, tag=f"w1f_{mt}", name="w1")
                for mt in range(WG)
            ]
            PG = 2
            w1_tiles = [None, None] + [
                cpool.tile([P, PG, KD, P], XDT, tag=f"w1_{g}", name="w1")
                for g in range(2, MT // PG)
            ]
            emit_x_load(0)
            for mt in range(WG):
                nc.sync.dma_start(w1_first[mt][:], w1_r[:, mt])
            for g in range(2, MT // PG):
                nc.sync.dma_start(w1_tiles[g][:], w1_r[:, g * PG:(g + 1) * PG])

            def w1_slice(mt, ksl):
                if mt < WG:
                    return w1_first[mt][:, ksl, :]
                return w1_tiles[mt // PG][:, mt % PG, ksl, :]
            ones_kcol = cpool.tile([P, 1], BF16, tag="ones_kcol", name="ones_kcol")
            nc.vector.memset(ones_kcol[:], 1.0)
            # PE p-state warm-up: ~24 trivial matmuls that only depend on the
            # ones_kcol memset run during the initial DMA wait, so the PE is
            # already at max clock when the first real mm1 group issues.
            if fast:
                warm_ps = ps_s.tile([P, C // P], F32, tag="sst", name="warm")
                for _ in range(120):
                    nc.tensor.matmul(warm_ps[:1, :1], lhsT=ones_kcol[:],
                                     rhs=ones_kcol[:], start=True, stop=True,
                                     skip_group_check=True)
            ones_krow_f = cpool.tile([1, P], F32, tag="ones_krow_f", name="ones_krow_f")
            nc.vector.memset(ones_krow_f[:], 1.0)
            eps_sb = cpool.tile([P, 1], F32, tag="eps", name="eps")
            nc.vector.memset(eps_sb[:], EPS)
            if b1_nz:
                b1_sb = cpool.tile([1, H], BF16, tag="b1", name="b1")
                nc.sync.dma_start(b1_sb[:], b1_d.rearrange("h x -> x h"))
                ones_row = cpool.tile([1, C], BF16, tag="ones_row", name="ones_row")
                nc.vector.memset(ones_row[:], 1.0)
            if ns_nb_nz:
                nsc_sb = cpool.tile([P, MT], F32, tag="nsc", name="nsc")
                nc.sync.dma_start(nsc_sb[:], nsc_d.rearrange("(mt p) x -> p mt x", p=P)[:, :, 0])
                nbs_sb = cpool.tile([P, MT], F32, tag="nbs", name="nbs")
                nc.sync.dma_start(nbs_sb[:], nbs_d.rearrange("(mt p) x -> p mt x", p=P)[:, :, 0])
            if b2_nz:
                b2_sb = cpool.tile([P, DT], F32, tag="b2", name="b2")
                nc.sync.dma_start(b2_sb[:], b2_d.rearrange("(dt p) x -> p dt x", p=P)[:, :, 0])
            # w2 loads are deferred into chunk 0's mm1 phase: issuing them at
            # t=0 steals HBM bandwidth from the w1/x transfers that gate mm1
            # startup, while w2 isn't needed until the first mm2 (~27us in)
            w2_sb = cpool.tile([P, KH, D], W2DT, tag="w2", name="w2")

            for c in range(NCH):
                xt = x_tiles[c]
                hn = [
                    hnpool.tile([P, 2, C], F8 if mm2_fp8 else BF16,
                                tag=f"hn{j}", name="hn")
                    for j in range(HP)
                ]
                hflat = hgenpool.tile([P, KH, C], F32, tag="hflat", name="hflat") if ns_nb_nz else None
                # mm1: 16 row-tile groups. ACT squares each PSUM tile (bf16);
                # the 16 squared tiles are folded with two sequential
                # accumulators — even leaves on DVE, odd leaves on the
                # otherwise-idle GpSimd — plus one final merge add, so the
                # post-mm1 tail is a single square + one add. Relus (the hn
                # eviction mm2 consumes) go 12 on DVE / 4 on ACT to keep
                # both engines under the PE's per-chunk budget.
                sq_tiles = [None] * MT
                accA = accB = None
                for mt in range(MT):
                    ph = ps_h.tile([P, C], F32, tag="ph", name="ph")
                    if mm1_fp8:
                        for kt in range(0, KD, 2):
                            nc.tensor.matmul(
                                ph[:], lhsT=w1_slice(mt, slice(kt, kt + 2)),
                                rhs=xt[:, kt // 2], start=(kt == 0),
                                stop=(kt == KD - 2 and not b1_nz),
                                perf_mode=mybir.MatmulPerfMode.DoubleRow,
                            )
                    else:
                        for kt in range(KD):
                            nc.tensor.matmul(
                                ph[:], lhsT=w1_slice(mt, kt),
                                rhs=xt[:, kt // 2, kt % 2, :], start=(kt == 0),
                                stop=(kt == KD - 1 and not b1_nz),
                            )
                    if b1_nz:
                        nc.tensor.matmul(
                            ph[:], lhsT=b1_sb[:, mt * P:(mt + 1) * P], rhs=ones_row[:],
                            start=False, stop=True, skip_group_check=True,
                        )
                    # mt 7/15 relu on ACT (PSUM access, keeps DVE under
                    # budget). For mt=15 the square goes FIRST: sq15 gates the
                    # whole rstd chain while relu15 only feeds mm2's last
                    # k-pair (~13us of slack).
                    sq = sqpool.tile([P, C], BF16, tag="sq4", name="sq4")
                    sq_tiles[mt] = sq

                    def emit_hn_evict():
                        if ns_nb_nz:
                            nc.vector.tensor_copy(hflat[:, mt, :], ph[:])
                        elif mt % 8 == 7:
                            nc.scalar.activation(
                                hn[mt // 2][:, mt % 2, :], ph[:],
                                mybir.ActivationFunctionType.Relu,
                            )
                        else:
                            nc.vector.tensor_scalar_max(
                                hn[mt // 2][:, mt % 2, :], ph[:], 0.0)

                    if mt == MT - 1:
                        nc.scalar.square(sq[:], ph[:])
                        emit_hn_evict()
                    else:
                        emit_hn_evict()
                        nc.scalar.square(sq[:], ph[:])
                    if c == 0:
                        # deferred w2 loads, one k-tile per mm1 row-tile
                        nc.sync.dma_start(w2_sb[:, mt, :], w2_r[:, mt, :])
                    # even leaves 0-14 fold on the (otherwise idle, slower)
                    # GpSimd which goes quiet by mm1's end; odd leaves 1-13 on
                    # DVE; the two accumulators merge at ~mm1 end and sq15
                    # folds LAST, so only one DVE add trails the final square
                    # and the stats matmuls never stall the PE.
                    if mt == 2:
                        accB = sqpool.tile([P, C], BF16, tag="accB", name="accB")
                        nc.gpsimd.tensor_add(accB[:], sq_tiles[0][:], sq_tiles[2][:])
                    elif mt == 3:
                        accA = sqpool.tile([P, C], BF16, tag="accA", name="accA")
                        nc.vector.tensor_add(accA[:], sq_tiles[1][:], sq_tiles[3][:])
                    elif mt > 3 and mt < 15 and mt % 2 == 0:
                        nc.gpsimd.tensor_add(accB[:], accB[:], sq[:])
                    elif mt > 3 and mt < 15:
                        nc.vector.tensor_add(accA[:], accA[:], sq[:])
                merge0 = sqpool.tile([P, C], BF16, tag="mrg", name="mrg")
                nc.vector.tensor_add(merge0[:], accA[:], accB[:])
                hacc_bf = sqpool.tile([P, C], BF16, tag="hacc", name="hacc")
                nc.vector.tensor_add(hacc_bf[:], merge0[:], sq_tiles[15][:])

                if c + 1 < NCH:
                    emit_x_load(c + 1)

                def emit_stats_head(ss):
                    # (general path only) ss[1, C] = sum_p hacc_bf -> std ->
                    # rstd (row ops; the [1, C] reciprocal is a slow
                    # single-lane DVE op ~3.3us).
                    nc.tensor.matmul(ss[:1, :], lhsT=ones_kcol[:], rhs=hacc_bf[:],
                                     start=True, stop=True, skip_group_check=True)
                    std = rowpool.tile([1, C], F32, tag="std", name="std")
                    nc.scalar.activation(
                        std[:], ss[:1, :], mybir.ActivationFunctionType.Sqrt,
                        bias=eps_sb[:1, :], scale=1.0 / H,
                    )
                    rstd = rowpool.tile([1, C], F32, tag="rstd", name="rstd")
                    nc.vector.reciprocal(rstd[:], std[:])
                    return rstd

                def emit_rb(rstd):
                    # broadcast rstd across partitions on the (idle) GpSimd
                    rb = rbpool.tile([P, C], F32, tag="rb", name="rb")
                    nc.gpsimd.partition_broadcast(rb[:], rstd[:], channels=P)
                    return rb

                if ns_nb_nz:
                    # general path: hn = relu(((h-mu)*rstd)*nsc + nbs)
                    ss = ps_s.tile([P, C], F32, tag="small", name="small")
                    rstd = emit_stats_head(ss)
                    rb = emit_rb(rstd)
                    for mt in range(MT):
                        tmp = rbpool.tile([P, C], F32, tag="tmpn", name="tmpn")
                        nc.vector.tensor_mul(tmp[:], hflat[:, mt, :], rb[:])
                        nc.scalar.activation(
                            hn[mt // 2][:, mt % 2, :], tmp[:],
                            mybir.ActivationFunctionType.Relu,
                            bias=nbs_sb[:, mt, None], scale=nsc_sb[:, mt, None],
                        )

                    for dt in range(DT):
                        py = ps_y.tile([P, C], F32, tag="py", name="py")
                        for kt in range(KH):
                            nc.tensor.matmul(
                                py[:], lhsT=w2_sb[:, kt, dt * P:(dt + 1) * P],
                                rhs=hn[kt // 2][:, kt % 2, :], start=(kt == 0),
                                stop=(kt == KH - 1),
                            )
                        ysb = ypool.tile([P, C], F32, tag="y", name="y")
                        nc.vector.tensor_copy(ysb[:], py[:])
                        if b2_nz:
                            nc.vector.tensor_scalar_add(ysb[:], ysb[:], b2_sb[:, dt, None])
                        nc.sync.dma_start(yT_r[:, dt, c * C:(c + 1) * C], ysb[:])
                else:
                    # fast path: mm2 runs "transposed" — hn column-slices are
                    # the stationary operand, W2 the moving one, so the PSUM
                    # output is [c-subtile(128), d] and rstd is a per-PARTITION
                    # scalar. The stats reduce to 4 tiny [128,1] ones-matmuls
                    # + [128,4] sqrt/reciprocal (sub-us, vs ~4us for the
                    # single-lane [1,C] row chain), evictions are
                    # tensor_scalar_muls, and no partition-broadcast is
                    # needed. rstd > 0 commutes with relu so it can be
                    # applied after mm2; W2's host-side fp8 scale S2 is
                    # folded into the Sqrt scale: std' = S2*std, so the
                    # eviction multiply by 1/std' also divides out S2. The
                    # x/W1 scales cancel automatically (rstd is computed
                    # from h' itself).
                    CS = C // P   # 4 column-subtiles per chunk
                    DH = D // C   # 2 moving halves of D per (cs)
                    sqrt_scale = (S2 * S2 / H) if mm2_fp8 else (1.0 / H)
                    pys = [[None] * DH for _ in range(CS)]

                    def y_mms(cs, dh):
                        pys[cs][dh] = ps_y.tile([P, C], F32, tag="py", name="py")
                        if mm2_fp8:
                            for kt in range(0, KH, 2):
                                nc.tensor.matmul(
                                    pys[cs][dh][:],
                                    lhsT=hn[kt // 2][:, :, cs * P:(cs + 1) * P],
                                    rhs=w2_sb[:, kt:kt + 2, dh * C:(dh + 1) * C],
                                    start=(kt == 0), stop=(kt == KH - 2),
                                    perf_mode=mybir.MatmulPerfMode.DoubleRow,
                                )
                        else:
                            for kt in range(KH):
                                nc.tensor.matmul(
                                    pys[cs][dh][:],
                                    lhsT=hn[kt // 2][:, kt % 2, cs * P:(cs + 1) * P],
                                    rhs=w2_sb[:, kt, dh * C:(dh + 1) * C],
                                    start=(kt == 0), stop=(kt == KH - 1),
                                )

                    def emit_stats_t():
                        # ss_t[:, cs] = sum_h h'^2 for column-subtile cs
                        ss_t = ps_s.tile([P, CS], F32, tag="sst", name="sst")
                        for cs in range(CS):
                            nc.tensor.matmul(
                                ss_t[:, cs:cs + 1],
                                lhsT=hacc_bf[:, cs * P:(cs + 1) * P],
                                rhs=ones_kcol[:], start=True, stop=True,
                                skip_group_check=True,
                            )
                        std_t = rowpool.tile([P, CS], F32, tag="stdt", name="stdt")
                        nc.scalar.activation(
                            std_t[:], ss_t[:],
                            mybir.ActivationFunctionType.Sqrt,
                            bias=eps_sb[:], scale=sqrt_scale,
                        )
                        rstd_t = rowpool.tile([P, CS], F32, tag="rstdt", name="rstdt")
                        nc.vector.reciprocal(rstd_t[:], std_t[:])
                        return rstd_t

                    def y_evict(cs, dh, rstd_t):
                        ysb = ypool.tile([P, C], BF16, tag="y", name="y")
                        nc.vector.tensor_scalar_mul(
                            ysb[:], pys[cs][dh][:], rstd_t[:, cs, None]
                        )
                        nc.sync.dma_start(
                            y_r[:, c * CS + cs, dh * C:(dh + 1) * C], ysb[:]
                        )

                    y_mms(0, 0)
                    emitted_stats = False
                    rstd_t = None
                    for i, (cs, dh) in enumerate(
                        [(cs, dh) for cs in range(CS) for dh in range(DH)][1:]
                    ):
                        y_mms(cs, dh)
                        if not emitted_stats:
                            rstd_t = emit_stats_t()
                            emitted_stats = True
                            y_evict(0, 0, rstd_t)
                        else:
                            prev = (cs * DH + dh) - 1
                            y_evict(prev // DH, prev % DH, rstd_t)
                    y_evict(CS - 1, DH - 1, rstd_t)

    nc.compile()
    return nc


# ------------------------------------------------------------ host logic ---
def _route(x0, ln_scale, ln_bias, Wr, br):
    """LayerNorm -> relu -> router logits -> top-2 -> gates (float64 math)."""
    x = x0.astype(np.float64)
    mu = x.mean(axis=-1, keepdims=True)
    var = np.square(x - mu).mean(axis=-1, keepdims=True)
    xn = (x - mu) / np.sqrt(var + EPS)
    xn = xn * ln_scale.astype(np.float64) + ln_bias.astype(np.float64)
    np.maximum(xn, 0.0, out=xn)
    logits = xn @ Wr.astype(np.float64) + br.astype(np.float64)

    n = logits.shape[0]
    rows = np.arange(n)
    i0 = np.argmax(logits, axis=1)
    l0 = logits[rows, i0]
    tmp = logits.copy()
    tmp[rows, i0] = -np.inf
    i1 = np.argmax(tmp, axis=1)
    l1 = tmp[rows, i1]
    # softmax over (l0, l1); l0 >= l1
    e1 = np.exp(l1 - l0)
    g0 = 1.0 / (1.0 + e1)
    g1 = e1 / (1.0 + e1)
    top_idx = np.stack([i0, i1], axis=1).astype(np.int64)
    gates = np.stack([g0, g1], axis=1)
    return xn.astype(np.float32), top_idx, gates


def _positions(top_idx):
    """Capacity positions: running per-expert count in token-major slot order."""
    eidx = top_idx.reshape(-1)
    nk = eidx.shape[0]
    oh = (eidx[:, None] == np.arange(E)[None, :]).astype(np.int64)
    pos = np.cumsum(oh, axis=0)[np.arange(nk), eidx] - 1
    mask = pos < CAP
    pos_c = np.minimum(pos, CAP - 1)
    return eidx, pos, pos_c, mask


def kernel(**inputs):
    x0 = np.asarray(inputs["x0"], np.float32)
    ln_scale = np.asarray(inputs["ln_scale"], np.float32)
    ln_bias = np.asarray(inputs["ln_bias"], np.float32)
    Wr = np.asarray(inputs["Wr"], np.float32)
    br = np.asarray(inputs["br"], np.float32)
    W1 = np.asarray(inputs["W1"], np.float32)
    b1 = np.asarray(inputs["b1"], np.float32)
    n_scale = np.asarray(inputs["n_scale"], np.float32)
    n_bias = np.asarray(inputs["n_bias"], np.float32)
    W2 = np.asarray(inputs["W2"], np.float32)
    b2 = np.asarray(inputs["b2"], np.float32)

    # ---- host routing + dispatch ---------------------------------------
    xn, top_idx, gates = _route(x0, ln_scale, ln_bias, Wr, br)
    eidx, pos, pos_c, mask = _positions(top_idx)

    tok_of_slot = np.repeat(np.arange(N), TOPK)
    keep = mask
    expert_inputs = np.zeros((E, CAP, D), np.float32)
    expert_inputs[eidx[keep], pos[keep]] = xn[tok_of_slot[keep]]

    # ---- build / fetch compiled program --------------------------------
    b1_nz = bool(np.any(b1))
    ns_nb_nz = bool(np.any(n_scale != 1.0) or np.any(n_bias))
    b2_nz = bool(np.any(b2))
    # fp8 DoubleRow only wired up for the fast path (all-zero biases /
    # identity norm, which is what this problem's inputs always are)
    fast = not (b1_nz or ns_nb_nz or b2_nz)
    mm1_fp8 = fast and VARIANT in ("full8", "mix8")
    mm2_fp8 = fast and VARIANT == "full8"
    flags = (b1_nz, ns_nb_nz, b2_nz, mm1_fp8, mm2_fp8)
    if flags not in _nc_cache:
        _nc_cache[flags] = _build(flags)
    nc = _nc_cache[flags]

    def to_f8(a):
        return np.clip(a, -F8CLIP, F8CLIP).astype(npf8)

    XP = KD // 2

    def tile_x(xT):
        # [D, CAP] -> [NCH, XP, P, 2, C]: per (chunk, k-pair) tiles whose
        # per-partition rows are contiguous 2*C runs (efficient DMA lines)
        a = xT.reshape(XP, 2, P, NCH, C)          # [j, v, p, c, cc]
        return np.ascontiguousarray(a.transpose(3, 0, 2, 1, 4))

    def tile_w1(w1p):
        # [D, H] -> [MT, P, KD, P]: per mm1-row-tile slices, contiguous rows
        a = w1p.reshape(KD, P, MT, P)             # [ko, p, mt, i]
        return np.ascontiguousarray(a.transpose(2, 1, 0, 3))

    # ---- per-core inputs ----------------------------------------------
    in_maps = []
    for e in range(E):
        # Fold the LayerNorm mean over H into the weights: x @ W1' = h - mu.
        w1p = W1[e].astype(np.float64)
        w1p = w1p - w1p.mean(axis=1, keepdims=True)
        xTe = np.ascontiguousarray(expert_inputs[e].T)
        m = {
            "xT": tile_x(to_f8(xTe) if mm1_fp8 else xTe.astype(npbf16)),
            "w1": tile_w1(to_f8(w1p * S1) if mm1_fp8 else w1p.astype(npbf16)),
            "w2": to_f8(W2[e] * S2) if mm2_fp8 else W2[e].astype(npbf16),
        }
        if b1_nz:
            b1p = b1[e].astype(np.float64)
            b1p = b1p - b1p.mean()
            m["b1"] = b1p.astype(npbf16)[:, None]
        if ns_nb_nz:
            m["nsc"] = n_scale[e].astype(np.float32)[:, None]
            m["nbs"] = n_bias[e].astype(np.float32)[:, None]
        if b2_nz:
            m["b2"] = b2[e].astype(np.float32)[:, None]
        in_maps.append(m)

    res = run_bass_kernel_spmd(nc, in_maps, core_ids=list(range(E)))

    # ---- combine -------------------------------------------------------
    w = (gates.astype(np.float32) * mask.reshape(N, TOPK))
    pos2 = pos_c.reshape(N, TOPK)
    mix = np.zeros((N, D), np.float32)
    if fast:
        y_all = np.stack([res.results[e]["y"] for e in range(E)])  # [E, CAP, D] bf16
        for k in range(TOPK):
            mix += y_all[top_idx[:, k], pos2[:, k], :].astype(np.float32) * w[:, k:k + 1]
    else:
        yT_all = np.stack([res.results[e]["yT"] for e in range(E)])  # [E, D, CAP]
        for k in range(TOPK):
            mix += yT_all[top_idx[:, k], :, pos2[:, k]] * w[:, k:k + 1]
    return x0 + mix

